# revision 22
# baseline (speedup 1.0000x reference)
"""Two-layer GAT on 8 Trainium2 NeuronCores (Bass/Tile).

Strategy (dst-sharded graph parallel):
  - Self-loops appended, edges sorted by destination; each core owns 1250
    consecutive dst nodes (10 super-tiles of 128 dsts). Per-dst softmax and
    the scatter-sum are device-local by construction.
  - Phase A (replicated): every core computes h1 = x @ W1 into an HBM table
    of f16 rows [1024 feats (c-major) | 8 f32 a_s | pad] (2304 B), plus a
    small a_d score table stab[N, 8] f32.
  - Phase B1 per super-tile: ONE dma_gather of source rows (features + a_s
    together); per-edge a_d comes from a transposed-one-hot matmul against
    the dense local a_d block (no second gather); exp on ACT; per-edge
    scaling of gathered rows via a single broadcast DVE multiply; scatter-sum
    via one-hot matmuls in PSUM; denominators via a matmul against the exp
    values themselves.
  - h2 = ELU(out1) @ W2 per shard -> tpl rows [64 h2 | a_s2 | a_d2 | pad]
    f16 (256 B); AllGather -> tpg; phase B2 repeats the edge pass for layer
    2 with ONE gather per super-tile.
"""
import sys

sys.path.insert(0, "/opt/trn_rl_repo")

import numpy as np

import concourse.bacc as bacc
import concourse.mybir as mybir
from concourse import hw_specs as _hw_specs
from concourse import tile as tile_mod
from concourse.bass_utils import run_bass_kernel_spmd
from concourse.tile import TileContext
from concourse.vector_clock import ScopedClock

# The tile scheduler plans engine overlap from CoreSim's cost model. Its
# SWDGE figure (0.34 ns/descriptor) underestimates measured dma_gather
# generation (~7.5 ns/idx on the Q7s) ~20x, so the scheduler fences the
# gathers behind the previous super-tile's compute instead of hoisting
# them. Calibrate to the measured rate so it pipelines them properly.
_hw_specs.TRN2Spec.SWDGE_NS_PER_DESCRIPTOR = 7.5

# ---------------------------------------------------------------- constants
N, E, FIN = 10000, 160000, 256
H1, C1, C2 = 8, 128, 64
D1 = H1 * C1                      # 1024
NEG = 0.2
NCORES = 8
NDST = N // NCORES                # 1250 dsts per core
STD = 128                         # dsts per super-tile
NST = (NDST + STD - 1) // STD     # 10
ROW1 = 1152                       # f16 slots: 1024 feats | 16 (8xf32 a_s) | pad
ROW2 = 128                        # f16: 64 h2 | a_s2 | a_d2 | pad
MCH = 79                          # node chunks of 128 (79*128 = 10112)
NPAD = MCH * 128

f16, f32 = np.float16, np.float32

# ------------------------------------------------- walrus 1-wait workaround


def _wait_cap(inst) -> int:
    return 2 if isinstance(inst, mybir.InstEventSemaphore) else 1


def _pop_appended(nc, inst):
    for f in nc.m.functions:
        for bb in f.blocks:
            if bb.instructions and bb.instructions[-1] is inst:
                bb.instructions.pop()
                return
    for f in nc.m.functions:
        for bb in f.blocks:
            if inst in bb.instructions:
                bb.instructions.remove(inst)
                return


def legalize_waits(nc):
    """This walrus build accepts one sync wait per instruction (two for
    EventSemaphore); hoist excess waits onto same-engine nops."""
    for f in nc.m.functions:
        for bb in f.blocks:
            new_insts = []
            for inst in list(bb.instructions):
                si = inst.sync_info
                waits = list(si.on_wait) if si is not None and si.on_wait else []
                cap = _wait_cap(inst)
                if len(waits) > cap:
                    si.on_wait = waits[:cap]
                    for w in waits[cap:]:
                        nop = nc.engines[inst.engine].nop()
                        nop.ins.sync_info = mybir.SyncInfo(on_wait=[w], on_update=[])
                        _pop_appended(nc, nop.ins)
                        new_insts.append(nop.ins)
                new_insts.append(inst)
            bb.instructions[:] = new_insts


def _patched_drain_and_barrier(self, tick_clock, wait_clock):
    nc = self.nc
    drain_inst = nc.sync.drain()
    wait_clock.add_sem_waits(
        drain_inst.ins, ScopedClock({None: tick_clock.global_clock})
    )
    si = drain_inst.ins.sync_info
    waits = list(si.on_wait) if si is not None and si.on_wait else []
    if len(waits) > 1:
        si.on_wait = waits[:1]
        bb = nc.cur_bb.bb
        for w in waits[1:]:
            nop = nc.sync.nop()
            nop.ins.sync_info = mybir.SyncInfo(on_wait=[w], on_update=[])
        insts = bb.instructions
        insts.remove(drain_inst.ins)
        insts.append(drain_inst.ins)

    nc.all_engine_barrier()
    assert self.sems is not None
    popped = nc._tile_sem_poison_stack.pop()
    assert popped is self._sem_poison
    nc.clear_and_free_semaphores(list(self.sems.allocated().values()))
    nc.all_engine_barrier()


tile_mod.TileContext._drain_and_barrier = _patched_drain_and_barrier

# ---------------------------------------------------------------- host prep


def _edge_struct(edge_index):
    src = np.concatenate([edge_index[0], np.arange(N, dtype=np.int64)])
    dst = np.concatenate([edge_index[1], np.arange(N, dtype=np.int64)])
    order = np.argsort(dst, kind="stable")
    src_s = src[order].astype(np.int32)
    dst_s = dst[order].astype(np.int32)

    marks = [k * NDST + s * STD for k in range(NCORES) for s in range(NST)]
    marks.append(N)
    bounds = np.searchsorted(dst_s, np.asarray(marks), side="left")
    cnt = np.diff(bounds)
    T = int(np.max((cnt + 127) // 128))
    NIDX = T * 128

    src16 = np.zeros((NCORES, NST, NIDX), np.int16)   # pads -> row 0
    S = np.zeros((NCORES, NST, NIDX, 128), f16)
    for k in range(NCORES):
        for s in range(NST):
            lo, hi = bounds[k * NST + s], bounds[k * NST + s + 1]
            n = hi - lo
            src16[k, s, :n] = src_s[lo:hi]
            dloc = dst_s[lo:hi] - (k * NDST + s * STD)
            S[k, s, np.arange(n), dloc] = 1.0
    S4 = S.reshape(NCORES, NST, T, 128, 128)
    # p-major contiguous loads: Sp[k,s,p,t,m] (one-hot by dst for edge t*128+p)
    Sp = np.ascontiguousarray(np.transpose(S4, (0, 1, 3, 2, 4)))
    Sp = Sp.reshape(NCORES, NST, 128, T * 128)
    # transposed one-hot: St[k,s,d,t,e] = S4[k,s,t,e,d]
    St = np.ascontiguousarray(np.transpose(S4, (0, 1, 4, 2, 3)))
    St = St.reshape(NCORES, NST, 128, T * 128)

    def wrap(idx):  # [NCORES, NST, NIDX] -> dma_gather 16-wrap, 8 replicas
        out = np.zeros((NCORES, NST, 128, T * 8), np.int16)
        i = np.arange(NIDX)
        for rep in range(8):
            out[:, :, 16 * rep + (i % 16), i // 16] = idx
        return out

    return wrap(src16), Sp, St, T


def _host_params(x, W1, att_src1, att_dst1, b1, W2, att_src2, att_dst2, b2):
    x = np.asarray(x, f32)
    xT = np.zeros((FIN, NPAD), f16)
    xT[:, :N] = x.T.astype(f16)

    cs, hs = np.meshgrid(np.arange(C1), np.arange(H1), indexing="ij")
    perm = (hs * C1 + cs).ravel()          # new c-major pos (c*8+h) -> old h*128+c
    # c-major interleave: col (c*8 + h) <- W1[:, h*128 + c]
    W1i = np.asarray(W1, f32).astype(f16)[:, perm]

    W1_64 = np.asarray(W1, np.float64)
    Ws = np.stack(
        [W1_64[:, h * C1:(h + 1) * C1] @ np.asarray(att_src1, np.float64)[h]
         for h in range(H1)], axis=1)
    Wd = np.stack(
        [W1_64[:, h * C1:(h + 1) * C1] @ np.asarray(att_dst1, np.float64)[h]
         for h in range(H1)], axis=1)
    Wsd = Ws.astype(f16)                                     # [256, 8] (a_s only)

    # dense layer-1 a_d scores per (core, super-tile): computed host-side so
    # the SPMD program needs no core-dependent addressing
    ad_full = (np.asarray(x, np.float64) @ Wd)               # [N, 8]
    adl = np.zeros((NCORES, NST, 128, H1), f16)
    for k in range(NCORES):
        blk = ad_full[k * NDST:(k + 1) * NDST]
        adl[k, :, :, :].reshape(NST * 128, H1)[:NDST] = blk.astype(f16)

    W2_64 = np.asarray(W2, np.float64)
    w2s = W2_64 @ np.asarray(att_src2, np.float64)[0]
    w2d = W2_64 @ np.asarray(att_dst2, np.float64)[0]
    W2e = np.zeros((D1, 66), f16)
    W2e[:, 0:64] = np.asarray(W2, f32).astype(f16)
    W2e[:, 64] = w2s.astype(f16)
    W2e[:, 65] = w2d.astype(f16)
    W2e = W2e[perm]        # rows to c-major K order

    b1cm = np.zeros((128, D1), f32)
    b1cm[:] = np.asarray(b1, f32)[perm][None, :]
    b2r = np.zeros((128, C2), f32)
    b2r[:] = np.asarray(b2, f32)[None, :]

    eye = np.eye(128, dtype=f32)
    return dict(xT=xT, W1i=W1i, Wsd=Wsd, W2e=W2e, b1r=b1cm, b2r=b2r, eye=eye), adl


# ------------------------------------------------------------- bass program
_prog_cache = {}


def _build(T):
    dt = mybir.dt
    Alu = mybir.AluOpType
    Act = mybir.ActivationFunctionType
    NIDX = T * 128

    nc = bacc.Bacc("TRN2", target_bir_lowering=False, debug=False,
                   num_devices=NCORES)
    xT = nc.dram_tensor("xT", [FIN, NPAD], dt.float16, kind="ExternalInput")
    W1i = nc.dram_tensor("W1i", [FIN, D1], dt.float16, kind="ExternalInput")
    Wsd = nc.dram_tensor("Wsd", [FIN, 8], dt.float16, kind="ExternalInput")
    W2e = nc.dram_tensor("W2e", [D1, 66], dt.float16, kind="ExternalInput")
    b1r = nc.dram_tensor("b1r", [128, D1], dt.float32, kind="ExternalInput")
    b2r = nc.dram_tensor("b2r", [128, C2], dt.float32, kind="ExternalInput")
    eye = nc.dram_tensor("eye", [128, 128], dt.float32, kind="ExternalInput")
    idxs = nc.dram_tensor("idxs", [NST, 128, T * 8], dt.int16, kind="ExternalInput")
    Sp = nc.dram_tensor("Sp", [NST, 128, NIDX], dt.float16, kind="ExternalInput")
    St = nc.dram_tensor("St", [NST, 128, NIDX], dt.float16, kind="ExternalInput")
    adlt = nc.dram_tensor("adlt", [NST, 128, H1], dt.float16, kind="ExternalInput")

    tg = nc.dram_tensor("tg", [N, ROW1], dt.float16)
    tpl = nc.dram_tensor("tpl", [NDST, ROW2], dt.float16)
    tpg = nc.dram_tensor("tpg", [N, ROW2], dt.float16, addr_space="Shared")
    out = nc.dram_tensor("out", [NDST, C2], dt.float32, kind="ExternalOutput")

    with TileContext(nc) as tc:
        with tc.tile_pool(name="const", bufs=1) as cp:
            w1i_sb = cp.tile([128, 2, D1], dt.float16)
            nc.sync.dma_start(w1i_sb[:], W1i.ap().rearrange("(j p) c -> p j c", p=128))
            wsd_sb = cp.tile([128, 2, 8], dt.float16)
            nc.sync.dma_start(wsd_sb[:], Wsd.ap().rearrange("(j p) c -> p j c", p=128))
            w2e_sb = cp.tile([128, 8, 66], dt.float16)
            nc.sync.dma_start(w2e_sb[:], W2e.ap().rearrange("(j p) c -> p j c", p=128))
            b1_sb = cp.tile([128, D1], dt.float32)
            nc.sync.dma_start(b1_sb[:], b1r[:])
            b2_sb = cp.tile([128, C2], dt.float32)
            nc.sync.dma_start(b2_sb[:], b2r[:])
            eye_sb = cp.tile([128, 128], dt.float32)
            nc.sync.dma_start(eye_sb[:], eye[:])

            # ---------------- phase A: h1 table (feats + a_s) ----------------
            with (
                tc.tile_pool(name="xa", bufs=3) as xap,
                tc.tile_pool(name="ha", bufs=3) as hap,
                tc.tile_pool(name="pa", bufs=2, space="PSUM") as pap,
                tc.tile_pool(name="psca", bufs=2, space="PSUM") as pscp,
            ):
                for i in range(MCH):
                    rows = min(128, N - i * 128)  # 128, last chunk 16
                    xb = xap.tile([128, 2, 128], dt.float16, tag="xb")
                    nc.sync.dma_start(
                        xb[:],
                        xT.ap()[:, i * 128:(i + 1) * 128]
                        .rearrange("(j p) c -> p j c", p=128),
                    )

                    psc = pscp.tile([128, 8], dt.float32)
                    for j in range(2):
                        nc.tensor.matmul(psc[:], xb[:, j, :], wsd_sb[:, j, :],
                                         start=(j == 0), stop=(j == 1))

                    ph = pap.tile([128, D1], dt.float32)
                    for j in range(2):
                        for s0, s1 in ((0, 512), (512, 1024)):
                            nc.tensor.matmul(ph[:, s0:s1], xb[:, j, :],
                                             w1i_sb[:, j, s0:s1],
                                             start=(j == 0), stop=(j == 1))
                    h1s = hap.tile([128, ROW1], dt.float16, tag="h1s")
                    nc.scalar.activation(h1s[:, 0:D1], ph[:], Act.Copy)
                    nc.vector.tensor_copy(
                        h1s[:, D1:D1 + 16].bitcast(dt.float32), psc[:])
                    nc.vector.memset(h1s[:, D1 + 16:ROW1], 0.0)
                    nc.sync.dma_start(
                        tg.ap()[i * 128:i * 128 + rows, :], h1s[0:rows, :]
                    )

            # ---------------- phase B1: layer-1 edge pass --------------------
            with (
                tc.tile_pool(name="ixp", bufs=2) as ixp,
                tc.tile_pool(name="sp1", bufs=2) as sp1,
                tc.tile_pool(name="stp", bufs=2) as stp,
                tc.tile_pool(name="gp", bufs=2) as gp,
                tc.tile_pool(name="scp", bufs=2) as scp,
                tc.tile_pool(name="up", bufs=2, space="PSUM") as upp,
                tc.tile_pool(name="smallp", bufs=2, space="PSUM") as smp,
                tc.tile_pool(name="o1p", bufs=2) as o1p,
                tc.tile_pool(name="tps", bufs=2) as tpsp,
                tc.tile_pool(name="etp", bufs=2) as etp,
                tc.tile_pool(name="tpp", bufs=1, space="PSUM") as tpp,
            ):
                def b1_loads(s):
                    ix = ixp.tile([128, T * 8], dt.int16, tag="ix")
                    nc.sync.dma_start(ix[:], idxs.ap()[s])
                    st_sb = sp1.tile([128, T, 128], dt.float16, tag="st")
                    nc.sync.dma_start(
                        st_sb[:], Sp.ap()[s].rearrange("p (t m) -> p t m", m=128))
                    stT_sb = stp.tile([128, T, 128], dt.float16, tag="stT")
                    nc.scalar.dma_start(
                        stT_sb[:], St.ap()[s].rearrange("p (t m) -> p t m", m=128))
                    # dense local a_d rows for this super-tile (host-computed)
                    adl16 = scp.tile([128, 8], dt.float16, tag="adl16")
                    nc.scalar.dma_start(adl16[:], adlt.ap()[s])
                    g = gp.tile([128, T, ROW1], dt.float16, tag="g")
                    nc.gpsimd.dma_gather(g[:], tg.ap(), ix[:], NIDX, NIDX, ROW1,
                                         single_packet=False)
                    return st_sb, stT_sb, adl16, g

                cur = b1_loads(0)
                for s in range(NST):
                    nd = min(STD, NDST - s * STD)
                    st_sb, stT_sb, adl16, g = cur
                    if s + 1 < NST:
                        cur = b1_loads(s + 1)

                    # per-edge a_d via transposed one-hot matmul; denominators
                    # (d8) later share the same PSUM tile
                    pad8 = smp.tile([128, T * 8 + 8], dt.float32, tag="pad8")
                    ps_ad = pad8[:, 0:T * 8].rearrange("p (t c) -> p t c", c=8)
                    d8 = pad8[:, T * 8:T * 8 + 8]
                    for t in range(T):
                        nc.tensor.matmul(ps_ad[:, t, :], stT_sb[:, t, :],
                                         adl16[:], start=True, stop=True)

                    sc = scp.tile([128, T, 8], dt.float32, tag="sc")
                    nc.vector.tensor_tensor(
                        sc[:], g[:, :, D1:D1 + 16].bitcast(dt.float32), ps_ad[:],
                        Alu.add)
                    lr = scp.tile([128, T, 8], dt.float32, tag="lr")
                    nc.vector.tensor_scalar_mul(lr[:], sc[:], NEG)
                    nc.vector.tensor_max(lr[:], lr[:], sc[:])
                    exf = scp.tile([128, T, 8], dt.float32, tag="exf")
                    nc.scalar.activation(exf[:], lr[:], Act.Exp)
                    exh = scp.tile([128, T, 8], dt.float16, tag="exh")
                    nc.vector.tensor_copy(exh[:], exf[:])

                    # scale gathered rows in place by exp (per edge, per head),
                    # one t-slice at a time so DVE overlaps the PE matmuls
                    u = upp.tile([128, D1], dt.float32, tag="u")
                    for t in range(T):
                        g4 = g[:, t, :].rearrange("p (c o) -> p c o", o=H1)
                        exb = exh[:, t, :].unsqueeze(1).broadcast_to(
                            [128, ROW1 // 8, H1])
                        nc.vector.tensor_tensor(g4, g4, exb, Alu.mult)
                        for s0, s1 in ((0, 512), (512, 1024)):
                            nc.tensor.matmul(u[:, s0:s1], st_sb[:, t, :],
                                             g[:, t, s0:s1],
                                             start=(t == 0), stop=(t == T - 1))
                        nc.tensor.matmul(d8, st_sb[:, t, :], exh[:, t, :],
                                         start=(t == 0), stop=(t == T - 1))

                    rc = scp.tile([128, 8], dt.float32, tag="rc")
                    nc.vector.reciprocal(rc[:], d8)
                    o1 = o1p.tile([128, D1], dt.float32, tag="o1")
                    o1v = o1[:].rearrange("p (c o) -> p c o", o=H1)
                    uv = u[:].rearrange("p (c o) -> p c o", o=H1)
                    rcb = rc[:].unsqueeze(1).broadcast_to([128, C1, H1])
                    nc.vector.tensor_tensor(o1v, uv, rcb, Alu.mult)
                    nc.vector.tensor_add(o1[:], o1[:], b1_sb[:])
                    # ELU
                    r = o1p.tile([128, D1], dt.float32, tag="relu")
                    nc.scalar.activation(r[:], o1[:], Act.Relu)
                    nc.vector.tensor_sub(o1[:], o1[:], r[:])       # min(x, 0)
                    ee = o1p.tile([128, D1], dt.float32, tag="ee")
                    nc.scalar.activation(ee[:], o1[:], Act.Exp)
                    elu = o1p.tile([128, D1], dt.float32, tag="elu")
                    nc.vector.scalar_tensor_tensor(elu[:], ee[:], -1.0, r[:],
                                                   Alu.add, Alu.add)
                    # transpose for the h2 matmul
                    eluT = etp.tile([128, 8, 128], dt.float16, tag="eluT")
                    h2p = tpp.tile([128, 66], dt.float32, tag="h2p")
                    for j in range(8):
                        tp_ps = tpp.tile([128, 128], dt.float32, tag="tp")
                        nc.tensor.transpose(tp_ps[:], elu[:, j * 128:(j + 1) * 128],
                                            eye_sb[:])
                        nc.vector.tensor_copy(eluT[:, j, :], tp_ps[:])
                    for j in range(8):
                        nc.tensor.matmul(h2p[:], eluT[:, j, :], w2e_sb[:, j, :],
                                         start=(j == 0), stop=(j == 7))
                    tp_sb = tpsp.tile([128, ROW2], dt.float16, tag="tpsb")
                    nc.vector.tensor_copy(tp_sb[:, 0:66], h2p[:])
                    nc.vector.memset(tp_sb[:, 66:ROW2], 0.0)
                    nc.scalar.dma_start(
                        tpl.ap()[s * STD:s * STD + nd, :], tp_sb[0:nd, :]
                    )

                nc.gpsimd.collective_compute(
                    "AllGather", Alu.bypass,
                    ins=[tpl[:]], outs=[tpg[:]],
                    replica_groups=[list(range(NCORES))],
                )

            # ---------------- phase B2: layer-2 edge pass --------------------
            with (
                tc.tile_pool(name="ixp2", bufs=2) as ixp2,
                tc.tile_pool(name="sp2", bufs=2) as sp2,
                tc.tile_pool(name="stp2", bufs=2) as stp2,
                tc.tile_pool(name="g2p", bufs=2) as g2p,
                tc.tile_pool(name="sc2p", bufs=2) as sc2p,
                tc.tile_pool(name="u2p", bufs=2, space="PSUM") as u2pp,
                tc.tile_pool(name="sm2p", bufs=2, space="PSUM") as sm2p,
                tc.tile_pool(name="o2p", bufs=2) as o2p,
            ):
                def b2_loads(s):
                    nd = min(STD, NDST - s * STD)
                    ix = ixp2.tile([128, T * 8], dt.int16, tag="ix2")
                    nc.sync.dma_start(ix[:], idxs.ap()[s])
                    st_sb = sp2.tile([128, T, 128], dt.float16, tag="st2")
                    nc.sync.dma_start(
                        st_sb[:], Sp.ap()[s].rearrange("p (t m) -> p t m", m=128))
                    stT_sb = stp2.tile([128, T, 128], dt.float16, tag="stT2")
                    nc.scalar.dma_start(
                        stT_sb[:], St.ap()[s].rearrange("p (t m) -> p t m", m=128))
                    tpr = sc2p.tile([128, ROW2], dt.float16, tag="tpr")
                    if nd < 128:
                        nc.vector.memset(tpr[:], 0.0)
                    nc.scalar.dma_start(
                        tpr[0:nd, :], tpl.ap()[s * STD:s * STD + nd, :])
                    g2 = g2p.tile([128, T, ROW2], dt.float16, tag="g2")
                    nc.gpsimd.dma_gather(g2[:], tpg.ap(), ix[:], NIDX, NIDX, ROW2,
                                         single_packet=False)
                    return st_sb, stT_sb, tpr, g2

                cur2 = b2_loads(0)
                for s in range(NST):
                    nd = min(STD, NDST - s * STD)
                    st_sb, stT_sb, tpr, g2 = cur2
                    if s + 1 < NST:
                        cur2 = b2_loads(s + 1)

                    pd2 = sm2p.tile([128, T + 1], dt.float32, tag="pd2")
                    ps_ad2 = pd2[:, 0:T]
                    d2 = pd2[:, T:T + 1]
                    for t in range(T):
                        nc.tensor.matmul(ps_ad2[:, t:t + 1], stT_sb[:, t, :],
                                         tpr[:, 65:66], start=True, stop=True)

                    sc2 = sc2p.tile([128, T], dt.float32, tag="sc2")
                    nc.vector.tensor_tensor(sc2[:], g2[:, :, 64], ps_ad2,
                                            Alu.add)
                    l2 = sc2p.tile([128, T], dt.float32, tag="l2")
                    nc.vector.tensor_scalar_mul(l2[:], sc2[:], NEG)
                    nc.vector.tensor_max(l2[:], l2[:], sc2[:])
                    e2f = sc2p.tile([128, T], dt.float32, tag="e2f")
                    nc.scalar.activation(e2f[:], l2[:], Act.Exp)
                    e2h = sc2p.tile([128, T], dt.float16, tag="e2h")
                    nc.vector.tensor_copy(e2h[:], e2f[:])

                    u2 = u2pp.tile([128, C2], dt.float32, tag="u2")
                    for t in range(T):
                        e2b = e2h[:, t:t + 1].broadcast_to([128, ROW2])
                        nc.vector.tensor_tensor(g2[:, t, :], g2[:, t, :], e2b,
                                                Alu.mult)
                        nc.tensor.matmul(u2[:], st_sb[:, t, :], g2[:, t, 0:C2],
                                         start=(t == 0), stop=(t == T - 1))
                        nc.tensor.matmul(d2, st_sb[:, t, :], e2h[:, t:t + 1],
                                         start=(t == 0), stop=(t == T - 1))

                    rc2 = sc2p.tile([128, 1], dt.float32, tag="rc2")
                    nc.vector.reciprocal(rc2[:], d2)
                    o2 = o2p.tile([128, C2], dt.float32, tag="o2")
                    nc.vector.tensor_scalar_mul(o2[:], u2[:], rc2[:, 0:1])
                    nc.vector.tensor_add(o2[:], o2[:], b2_sb[:])
                    nc.sync.dma_start(out.ap()[s * STD:s * STD + nd, :], o2[0:nd, :])

    nc.compile()
    legalize_waits(nc)
    return nc


def _get_prog(T):
    if T not in _prog_cache:
        _prog_cache[T] = _build(T)
    return _prog_cache[T]


# ------------------------------------------------------------------ kernel
def kernel(x, edge_index, W1, att_src1, att_dst1, b1, W2, att_src2, att_dst2,
           b2, _run_kwargs=None):
    edge_index = np.asarray(edge_index)
    src16, Sp, St, T = _edge_struct(edge_index)
    params, adl = _host_params(x, W1, att_src1, att_dst1, b1, W2, att_src2,
                               att_dst2, b2)
    nc = _get_prog(T)

    in_maps = []
    for k in range(NCORES):
        m = dict(params)
        m["idxs"] = src16[k]
        m["Sp"] = Sp[k]
        m["St"] = St[k]
        m["adlt"] = adl[k]
        in_maps.append(m)

    res = run_bass_kernel_spmd(nc, in_maps, list(range(NCORES)),
                               **(_run_kwargs or {}))
    full = np.concatenate([res.results[k]["out"] for k in range(NCORES)], axis=0)
    kernel.last_results = res
    return full.astype(f32)
